# revision 26
# baseline (speedup 1.0000x reference)
"""Trainium2 Bass kernel for masked edge-softmax attention aggregation.

  score[j] = (inputs @ H_v)[j]
  E[i,j]   = exp(adj[i,j]*score[j]) if adj[i,j]!=0 else 0
  out      = (E @ inputs) / rowsum(E)

Staging (host, layout only): adj row-sharded over 8 cores, pre-transposed
to adjT [N, R] fp16; inputs+ones packed as aug image [128, NJ*W] fp16
(hv head); H_v replicated.

v2 strategy (batched exp): instead of one ACT Exp per j-block with a
per-partition scale (1227ns each, 79x = 96.9us ACT busy = the v1
ceiling), a DVE prescale l = sl*score (tensor_scalar, 4x mode, 386ns)
decouples the scale so Exp runs on multi-block group tiles
(82.3us + 185ns/op). Masking splits three ways per block to fit every
engine under the new ~85us ACT cap:
  - PE-correction blocks (most): mn = (sl>0)-1 via one dual-op
    tensor_scalar (386ns); numerator = e0@aug + mn@aug (exact
    cancellation for absent edges in fp32 PSUM).
  - DVE-apply blocks: m = (sl>0); e = e0*m (tensor_tensor 711ns).
  - Pool-apply blocks: same m; e = e0*m on gpsimd (2575ns) to shed load.
Applies/e-matmuls for group g are emitted AFTER group g+1's
prescales/maskgens (one-group lag), so a late Exp never stalls the DVE
stream that feeds the next Exp: ACT runs gap-free.

Epilogue is bank-staggered: the last group's matmuls are emitted
bank-by-bank, each PSUM bank's reciprocal+scale+output-DMA issued as
soon as that bank's accumulation closes.

Engine busy (cost model): ACT ~84.9 (cap) > DMA 78.6 > DVE ~77 >
PE ~71(mid-pstate worst) > Pool ~60.
"""

import os

import numpy as np

import concourse.bacc as bacc
import concourse.bass as bass
import concourse.mybir as mybir
import concourse.tile as tile
from concourse.bass_utils import run_bass_kernel_spmd

N = 10000
D = 128
NCORES = 8
R = N // NCORES          # 1250 rows per core
P = 128
NJ = (N + P - 1) // P    # 79 j-blocks, last has 16 rows
NI = (R + P - 1) // P    # 10 i-tiles, last has 98 rows
W = D + 1                # aug width (inputs | ones)

F32 = mybir.dt.float32
F16 = mybir.dt.float16
AF = mybir.ActivationFunctionType
ALU = mybir.AluOpType

GROUP_SIZES = [
    int(x)
    for x in os.environ.get(
        "GROUP_SIZES", "1," + ",".join(["2"] * 38) + ",1,1"
    ).split(",")
]
assert sum(GROUP_SIZES) == NJ, sum(GROUP_SIZES)
MAXG = max(GROUP_SIZES)

SLAB_BUFS = int(os.environ.get("SLAB_BUFS", "10"))
# apply variant per block: every POOL_EVERY-th eligible block -> Pool,
# every DVE_EVERY-th -> DVE, rest -> PE correction
POOL_EVERY = int(os.environ.get("POOL_EVERY", "5"))
DVE_EVERY = int(os.environ.get("DVE_EVERY", "4"))
APPLY_FIRST = int(os.environ.get("APPLY_FIRST", "10"))   # blocks < this are corr
APPLY_LAST = int(os.environ.get("APPLY_LAST", "74"))    # blocks >= this are corr
REDUCE_LEAD = int(os.environ.get("REDUCE_LEAD", "6"))
CHUNK_LEAD = int(os.environ.get("CHUNK_LEAD", "22"))
POOL_TWO_STAGE = int(os.environ.get("POOL_TWO_STAGE", "1"))

# first chunks tiny so block 0 starts ASAP
SCORE_CHUNKS = [(0, 2), (2, 3)] + [(5 * k, 5) for k in range(1, 15)] + [(75, 4)]


def _pb(b):
    return P if b < NJ - 1 else N - (NJ - 1) * P


def _ri(i):
    return P if i < NI - 1 else R - (NI - 1) * P


def _variant(b):
    """'c' = PE-correction, 'd' = DVE apply, 'p' = Pool apply."""
    if b < APPLY_FIRST or b >= APPLY_LAST:
        return "c"
    if POOL_EVERY and b % POOL_EVERY == POOL_EVERY - 1:
        return "p"
    if DVE_EVERY and b % DVE_EVERY == DVE_EVERY - 1:
        return "d"
    return "c"


def build_nc():
    nc = bacc.Bacc("TRN2", target_bir_lowering=False, debug=False, num_devices=NCORES)

    adjt = nc.dram_tensor("adjt_shard", [N, R], F16, kind="ExternalInput")
    aug_img = nc.dram_tensor("aug_img", [P, D + NJ * W], F16, kind="ExternalInput")
    out_s = nc.dram_tensor("out_shard", [R, D], F32, kind="ExternalOutput")

    groups = []
    b0 = 0
    for gs in GROUP_SIZES:
        groups.append((b0, gs))
        b0 += gs

    with tile.TileContext(nc) as tc:
        with (
            tc.tile_pool(name="const", bufs=1) as constp,
            tc.tile_pool(name="stmp", bufs=6) as stmpp,
            tc.tile_pool(name="slab", bufs=SLAB_BUFS) as slabp,
            tc.tile_pool(name="lg", bufs=int(os.environ.get("LG_BUFS", "3"))) as lgp,
            tc.tile_pool(name="eg", bufs=int(os.environ.get("EG_BUFS", "6"))) as egp,
            tc.tile_pool(name="mask", bufs=12) as maskp,
            tc.tile_pool(name="ework", bufs=8) as eworkp,
            tc.tile_pool(name="fix", bufs=10) as fixp,
            tc.tile_pool(name="psumacc", bufs=1, space="PSUM") as psumaccp,
        ):
            # ---------------- constants / prologue ----------------
            hv_aug_sb = constp.tile([P, D + NJ * W], F16)
            hv_sb = hv_aug_sb[:, 0:D]
            aug_sb = hv_aug_sb[:, D : D + NJ * W]
            aug3 = aug_sb.rearrange("p (b w) -> p b w", w=W)
            score_sb = constp.tile([P, NJ], F32)

            def load_aug_chunk(c0, nb, with_hv=False):
                if with_hv:
                    nc.sync.dma_start(
                        hv_aug_sb[:, 0 : D + nb * W], aug_img[:, 0 : D + nb * W]
                    )
                else:
                    nc.sync.dma_start(
                        aug_sb[:, c0 * W : (c0 + nb) * W],
                        aug_img[:, D + c0 * W : D + (c0 + nb) * W],
                    )

            stmps = {}

            def score_mult(ci, engine, two_stage=False):
                c0, nb = SCORE_CHUNKS[ci]
                stmp = stmpp.tile([P, 5 * D], F16, tag="stmp", name=f"stmp{ci}")
                hv_rep = (
                    hv_sb
                    .rearrange("p (o d) -> p o d", o=1)
                    .broadcast_to([P, nb, D])
                )
                s3 = stmp[:, 0 : nb * D].rearrange("p (b d) -> p b d", d=D)
                engine.tensor_tensor(s3, aug3[:, c0 : c0 + nb, 0:D], hv_rep, ALU.mult)
                if two_stage:
                    h = D // 2
                    engine.tensor_tensor(
                        s3[:, :, 0:h], s3[:, :, 0:h], s3[:, :, h:D], ALU.add
                    )
                stmps[ci] = (stmp, two_stage)

            def score_reduce(ci):
                c0, nb = SCORE_CHUNKS[ci]
                stmp, two_stage = stmps.pop(ci)
                dd = D // 2 if two_stage else D
                nc.vector.tensor_reduce(
                    score_sb[:, c0 : c0 + nb],
                    stmp[:, 0 : nb * D]
                    .rearrange("p (b d) -> p b d", d=D)[:, :, 0:dd],
                    axis=mybir.AxisListType.X,
                    op=ALU.add,
                )

            def load_slab(b):
                pb = _pb(b)
                sl = slabp.tile([P, R], F16, tag="slab", name=f"sl{b}")
                nc.sync.dma_start(sl[0:pb, :], adjt[b * P : b * P + pb, :])
                return sl

            # ACT Exp table warm-up off the first exp's critical path
            warm = constp.tile([1, 1], F32)
            nc.vector.memset(warm[:, :], 0.0)
            warm2 = constp.tile([1, 1], F32)
            nc.scalar.activation(warm2[:, :], warm[:, :], AF.Exp)

            slabs = {}
            load_aug_chunk(*SCORE_CHUNKS[0], with_hv=True)
            sl0 = slabp.tile([P, R], F16, tag="slab", name="sl0")
            h0 = R // 2
            nc.sync.dma_start(sl0[:, 0:h0], adjt[0:P, 0:h0])
            nc.sync.dma_start(sl0[:, h0:R], adjt[0:P, h0:R])
            slabs[0] = sl0
            score_mult(0, nc.vector, two_stage=True)
            score_reduce(0)
            load_aug_chunk(*SCORE_CHUNKS[1])
            slabs[1] = load_slab(1)
            load_aug_chunk(*SCORE_CHUNKS[2])
            slabs[2] = load_slab(2)

            # chunks 1-2: mult+fold inline on DVE between early fronts (no
            # Pool coupling on the critical early path); chunks 3+: Pool,
            # loaded far ahead; reduces just-in-time on DVE
            mult_at_block = {1: [(1, nc.vector)], 2: [(2, nc.vector)]}
            chunk_at_block = {}
            for ci in range(3, len(SCORE_CHUNKS)):
                chunk_at_block.setdefault(
                    max(0, SCORE_CHUNKS[ci][0] - CHUNK_LEAD), []
                ).append(ci)
            reduce_at_block = {}
            for ci in range(1, len(SCORE_CHUNKS)):
                blk = (
                    ci + 1 if ci <= 2
                    else max(4, SCORE_CHUNKS[ci][0] - REDUCE_LEAD)
                )
                reduce_at_block.setdefault(blk, []).append(ci)

            # PSUM accumulators: 10 i-tiles, 3 slots of W f32 per bank tile
            accs = [
                psumaccp.tile([P, 512], F32, tag=f"accb{t}", name=f"accb{t}")
                for t in range(4)
            ]

            def acc_ap(it, ri):
                t, s = divmod(it, 3)
                return accs[t][0:ri, s * 136 : s * 136 + W]

            # per-group l and e0 tiles (contiguous so Exp is one op)
            def group_tiles(gi, gs):
                lt = lgp.tile([P, MAXG * R], F16, tag="lg", name=f"lg{gi}")
                et = egp.tile([P, MAXG * R], F16, tag="eg", name=f"eg{gi}")
                return lt, et

            masks = {}    # b -> (m or mn tile, variant)
            einfo = {}    # b -> (e-tile/ap for matmuls, weights are e0 or applied e)

            def emit_block_front(b, lt, seg):
                """slab wait, prescale into lt segment, maskgen, mn-matmuls."""
                pb = _pb(b)
                for ci in chunk_at_block.get(b, ()):
                    load_aug_chunk(*SCORE_CHUNKS[ci])
                    score_mult(ci, nc.gpsimd, two_stage=bool(POOL_TWO_STAGE))
                for ci, eng in mult_at_block.get(b, ()):
                    score_mult(ci, eng, two_stage=True)
                for ci in reduce_at_block.get(b, ()):
                    score_reduce(ci)
                sl = slabs.pop(b) if b in slabs else load_slab(b)
                # prescale: l = sl * score[:, b] (block 0 in halves so the
                # first half-exp starts as soon as the first half-slab lands)
                if b == 0:
                    h = R // 2
                    for lo, hi in ((0, h), (h, R)):
                        nc.vector.tensor_scalar(
                            lt[0:pb, lo:hi],
                            sl[0:pb, lo:hi],
                            score_sb[0:pb, b : b + 1],
                            None,
                            ALU.mult,
                        )
                else:
                    nc.vector.tensor_scalar(
                        lt[0:pb, seg * R : seg * R + R],
                        sl[0:pb, :],
                        score_sb[0:pb, b : b + 1],
                        None,
                        ALU.mult,
                    )
                v = _variant(b)
                mt = maskp.tile([P, R], F16, tag="mask")
                if v == "c":
                    # mn = (sl>0) - 1 in one dual-op pass
                    nc.vector.tensor_scalar(
                        mt[0:pb, :], sl[0:pb, :], 0.0, 1.0, ALU.is_gt, ALU.subtract
                    )
                    # correction matmuls run early (independent of exp)
                    for it in range(NI):
                        ri = _ri(it)
                        nc.tensor.matmul(
                            acc_ap(it, ri),
                            mt[0:pb, it * P : it * P + ri],
                            aug3[0:pb, b, :],
                            start=(b == 0) and (it % 3 == 0),
                            stop=False,
                        )
                    masks[b] = (None, v)
                else:
                    nc.vector.tensor_scalar(
                        mt[0:pb, :], sl[0:pb, :], 0.0, None, ALU.is_gt
                    )
                    masks[b] = (mt, v)

            def apply_block(b, et, seg):
                """mask-apply (if needed); returns weights source (tile, base)."""
                pb = _pb(b)
                mt, v = masks.pop(b)
                if v == "c":
                    return et, seg * R
                ew = eworkp.tile([P, R], F16, tag="ework")
                eng = nc.gpsimd if v == "p" else nc.vector
                eng.tensor_tensor(
                    ew[0:pb, :], et[0:pb, seg * R : (seg + 1) * R], mt[0:pb, :], ALU.mult
                )
                return ew, 0

            def emit_block_back(b, et, seg):
                """apply (if needed) + e/e0 matmuls for a completed-exp block."""
                pb = _pb(b)
                wt, base = apply_block(b, et, seg)
                for it in range(NI):
                    ri = _ri(it)
                    nc.tensor.matmul(
                        acc_ap(it, ri),
                        wt[0:pb, base + it * P : base + it * P + ri],
                        aug3[0:pb, b, :],
                        start=False,
                        stop=False,
                    )

            # ---------------- main pipeline ----------------
            # corr blocks' e0-matmuls emit right after their group's exp
            # (PE-only, no DVE coupling); apply blocks lag one group so a
            # late exp can never stall the DVE prescale stream.
            pending_apply = []  # (b, et, seg) apply-variant blocks awaiting back
            last_g0, last_gs = groups[-1]
            last_et = None
            for gi, (g0, gs) in enumerate(groups):
                lt, et = group_tiles(gi, gs)
                for seg in range(gs):
                    emit_block_front(g0 + seg, lt, seg)
                is_last = gi == len(groups) - 1
                if gi == 0 and gs == 1:
                    h = R // 2
                    nc.scalar.activation(et[0:P, 0:h], lt[0:P, 0:h], AF.Exp)
                    nc.scalar.activation(et[0:P, h:R], lt[0:P, h:R], AF.Exp)
                elif not is_last:
                    nc.scalar.activation(
                        et[0:P, 0 : gs * R], lt[0:P, 0 : gs * R], AF.Exp
                    )
                if is_last:
                    last_et = et
                    last_lt = lt
                    for seg in range(gs):
                        assert _variant(g0 + seg) == "c", "last group must be corr"
                else:
                    for seg in range(gs):
                        b = g0 + seg
                        if _variant(b) == "c":
                            emit_block_back(b, et, seg)
                    ready, pending_apply = pending_apply, []
                    for b, pet_, pseg in ready:
                        emit_block_back(b, pet_, pseg)
                    for seg in range(gs):
                        b = g0 + seg
                        if _variant(b) != "c":
                            pending_apply.append((b, et, seg))

            # ---------------- epilogue (bank-staggered) ----------------
            # leftover applies from the second-to-last group
            for b, pet_, pseg in pending_apply:
                emit_block_back(b, pet_, pseg)
            pg0, pgs, pet = last_g0, last_gs, last_et
            osb = fixp.tile([P, NI * D], F32, tag="osb", bufs=1)
            osb3 = osb[:, :].rearrange("p (i d) -> p i d", d=D)

            def fixup_bank(t, queue=None):
                its = [it for it in range(NI) if it // 3 == t]
                ns = len(its)
                rec = fixp.tile([P, 4], F32, tag="rec")
                nc.vector.reciprocal(
                    rec[0:P, 0:ns],
                    accs[t][0:P, D : D + 1 + 136 * (ns - 1) : 136][0:P, 0:ns]
                    if ns > 1
                    else accs[t][0:P, D : D + 1],
                )
                for k, it in enumerate(its):
                    ri = _ri(it)
                    a = acc_ap(it, ri)
                    if it % 2 == 0:
                        nc.scalar.activation(
                            osb3[0:ri, it, :],
                            a[0:ri, 0:D],
                            AF.Copy,
                            bias=0.0,
                            scale=rec[0:ri, k : k + 1],
                        )
                    else:
                        nc.vector.tensor_scalar(
                            osb3[0:ri, it, :],
                            a[0:ri, 0:D],
                            rec[0:ri, k : k + 1],
                            None,
                            ALU.mult,
                        )
                it0 = its[0]
                q = queue if queue is not None else nc.sync
                rows = sum(_ri(it) for it in its)
                if rows == len(its) * P:
                    q.dma_start(
                        out_s[it0 * P : it0 * P + rows, :].rearrange(
                            "(i p) d -> p i d", p=P
                        ),
                        osb3[:, it0 : it0 + len(its), :],
                    )
                else:
                    nfull = rows // P
                    if nfull:
                        q.dma_start(
                            out_s[it0 * P : (it0 + nfull) * P, :].rearrange(
                                "(i p) d -> p i d", p=P
                            ),
                            osb3[:, it0 : it0 + nfull, :],
                        )
                    rpart = rows - nfull * P
                    q.dma_start(
                        out_s[(it0 + nfull) * P : (it0 + nfull) * P + rpart, :],
                        osb3[0:rpart, it0 + nfull, :],
                    )

            # last group (all corr): per-bank exp slice + matmuls, each
            # bank's fixup+DMA issued as soon as its accumulation closes
            for t in range(4):
                bank_its = [it for it in range(NI) if it // 3 == t]
                c0 = bank_its[0] * P
                c1 = bank_its[-1] * P + _ri(bank_its[-1])
                for seg in range(pgs):
                    nc.scalar.activation(
                        pet[0:P, seg * R + c0 : seg * R + c1],
                        last_lt[0:P, seg * R + c0 : seg * R + c1],
                        AF.Exp,
                    )
                for seg in range(pgs):
                    b = pg0 + seg
                    pb = _pb(b)
                    masks.pop(b, None)
                    for it in bank_its:
                        ri = _ri(it)
                        sl_ = it % 3
                        last_in_bank = (sl_ == 2) or (it == NI - 1)
                        nc.tensor.matmul(
                            acc_ap(it, ri),
                            pet[0:pb, seg * R + it * P : seg * R + it * P + ri],
                            aug3[0:pb, b, :],
                            start=False,
                            stop=(seg == pgs - 1) and last_in_bank,
                        )
                fixup_bank(t, queue=(nc.gpsimd if t % 2 else nc.sync))

    nc.compile()
    return nc


_NC = None


def _get_nc():
    global _NC
    if _NC is None:
        _NC = build_nc()
    return _NC


def _stage_inputs(inputs, adj, H_v):
    """Host-side layout staging: shard + transpose + fp16 + aug image."""
    inputs = np.asarray(inputs, dtype=np.float32)
    adj = np.asarray(adj, dtype=np.float32)
    H_v = np.asarray(H_v, dtype=np.float32)

    adj16t = np.ascontiguousarray(adj.astype(np.float16).T)  # [N, N]

    aug = np.zeros((P, D + NJ * W), dtype=np.float16)
    aug[:, 0:D] = H_v.reshape(1, D).astype(np.float16)
    inp16 = inputs.astype(np.float16)
    for b in range(NJ):
        pb = _pb(b)
        aug[0:pb, D + b * W : D + b * W + D] = inp16[b * P : b * P + pb, :]
        aug[0:pb, D + b * W + D] = np.float16(1.0)
    in_maps = [
        {
            "adjt_shard": np.ascontiguousarray(adj16t[:, c * R : (c + 1) * R]),
            "aug_img": aug,
        }
        for c in range(NCORES)
    ]
    return in_maps


def kernel(inputs, adj, H_v, _trace=False, _trace_kwargs=None):
    nc = _get_nc()
    in_maps = _stage_inputs(inputs, adj, H_v)
    kw = {}
    if _trace:
        kw = dict(trace=True, **(_trace_kwargs or {}))
    res = run_bass_kernel_spmd(nc, in_maps, list(range(NCORES)), **kw)
    if _trace:
        kernel._last_results = res
    outs = res.results
    return np.concatenate(
        [np.asarray(outs[c]["out_shard"], dtype=np.float32) for c in range(NCORES)],
        axis=0,
    )
